# revision 1
# baseline (speedup 1.0000x reference)
"""Single-head attention on 8 Trainium2 NeuronCores.

Problem: B=8, S=2048, WIDTH=1024, HEAD=64 single attention head.
Sharding: data-parallel over batch -- batch b runs on core b. No collectives.

Per-core pipeline (all matmuls in float32r -- full-rate fp32 on the PE):
  phase A: load x^T (host-pretransposed), project [Wk|Wq*scale] -> psum
           [K^T; Q^T] stacked [128, S]; Wv -> V^T [64, S].  Copy out with
           biases.  DMA-copy Q^T half down to partitions 0:64.  PE-transpose
           V^T into V chunks [128, 65] with a ones column appended.
  phase B: per q-block of 1024, per k-chunk of 128:
           scores^T = K^T_chunk.T @ Q^T  (PSUM [128, 1024])
           expS = exp(scores^T + mask_bias)   (ScalarE, from PSUM)
           O'^T += V'_chunk.T @ expS          (accumulate [65, 1024])
           row 64 of O'^T = softmax denominators (ones column of V').
           Transpose O'^T 128-col slices, normalize with per-partition
           reciprocal scale, DMA out rows.
"""

import os
from contextlib import ExitStack

import numpy as np

import concourse.bass as bass
import concourse.tile as tile
from concourse import mybir
from concourse.bass import ts

S = 2048
W = 1024
H = 64
N_CORES = 8
WC = W // 128  # 8 w-chunks
KC = S // 128  # 16 k-chunks
QB = 1024      # q-block size
NQB = S // QB  # 2
QT = QB // 128  # 8 q-tiles per block

F32 = mybir.dt.float32
F32R = mybir.dt.float32r
AF = mybir.ActivationFunctionType


def _emit(ctx, tc, xT, wkq, wv, bkq, bv, ident, mbias, vones, zeros64, out,
          rep="", probe=None, pack_scores=False):
    nc = tc.nc

    def pool(name, **kw):
        return ctx.enter_context(tc.tile_pool(name=name + rep, **kw))

    singles = pool("singles", bufs=1)
    wkq_sb = singles.tile([128, WC * 128], F32R)
    nc.scalar.dma_start(out=wkq_sb, in_=wkq)
    wv_sb = singles.tile([128, WC * H], F32R)
    nc.scalar.dma_start(out=wv_sb, in_=wv)
    bkq_sb = singles.tile([128, 1], F32)
    nc.scalar.dma_start(out=bkq_sb, in_=bkq)
    bv_sb = singles.tile([64, 1], F32)
    nc.scalar.dma_start(out=bv_sb, in_=bv)
    ident_sb = singles.tile([128, 128], F32R)
    nc.scalar.dma_start(out=ident_sb, in_=ident)
    mbias_sb = singles.tile([128, KC], F32)
    nc.scalar.dma_start(out=mbias_sb, in_=mbias)

    kq_sb = singles.tile([128, S], F32R)  # rows 0:64 = K^T, rows 64:128 = Q^T
    q_sb = singles.tile([64, S], F32R)    # Q^T copied down to partitions 0:64
    k2_sb = None
    if pack_scores:
        k2_sb = singles.tile([128, S], F32R, tag="k2sb" + rep)
    vT_sb = singles.tile([64, S], F32R)
    v_sb = singles.tile([128, KC, H + 1], F32R)  # V' chunks (+ones col)

    # ---------------- Phase A: projections ----------------
    with (
        tc.tile_pool(name="xp" + rep, bufs=3) as xp,
        tc.tile_pool(name="kqps" + rep, bufs=1, space="PSUM") as kqps_pool,
        tc.tile_pool(name="vtps" + rep, bufs=1, space="PSUM") as vtps_pool,
    ):
        kq_ps = kqps_pool.tile([128, S], F32)  # 4 banks
        vT_ps = vtps_pool.tile([64, S], F32)   # 4 banks
        NG = WC               # x load groups
        CPG = WC // NG        # w-chunks per group
        xTv = xT.rearrange("(g c p) s -> g p c s", c=CPG, p=128)
        for g in range(NG):
            xt = xp.tile([128, CPG, S], F32R)
            nc.sync.dma_start(out=xt, in_=xTv[g])
            for cc in range(CPG):
                c = g * CPG + cc
                for j in range(S // 512):
                    nc.tensor.matmul(
                        kq_ps[:, ts(j, 512)],
                        wkq_sb[:, ts(c, 128)],
                        xt[:, cc, ts(j, 512)],
                        start=(c == 0),
                        stop=(c == WC - 1),
                    )
                for j in range(S // 512):
                    nc.tensor.matmul(
                        vT_ps[:, ts(j, 512)],
                        wv_sb[:, ts(c, H)],
                        xt[:, cc, ts(j, 512)],
                        start=(c == 0),
                        stop=(c == WC - 1),
                    )
        # copy-outs split in halves so downstream consumers start early:
        # Q^T (DVE) feeds the q_sb partition-remap DMA; K^T + V^T on ACT.
        for hh in range(2):
            sl = slice(hh * (S // 2), (hh + 1) * (S // 2))
            nc.vector.tensor_scalar_add(
                kq_sb[64:128, sl], kq_ps[64:128, sl], bkq_sb[64:128, :]
            )
            nc.scalar.dma_start(out=q_sb[:, sl], in_=kq_sb[64:128, sl])
        for hh in range(2):
            sl = slice(hh * (S // 2), (hh + 1) * (S // 2))
            nc.scalar.activation(
                kq_sb[0:64, sl], kq_ps[0:64, sl], AF.Identity,
                bias=bkq_sb[0:64, :], scale=1.0,
            )
            if pack_scores:
                # K^T replicated into partitions 64:127 for row-half B
                nc.scalar.dma_start(out=k2_sb[64:128, sl], in_=kq_sb[0:64, sl])
        nc.scalar.activation(vT_sb, vT_ps, AF.Identity, bias=bv_sb, scale=1.0)
    nc.scalar.dma_start(
        out=v_sb[:, :, H : H + 1],
        in_=vones.rearrange("p (k one) -> p k one", one=1),
    )

    def emit_vtrans(ps_pool, tag):
        # V^T -> V natural chunks (PE transpose via identity, PSUM bounce)
        for t in range(KC):
            vp = ps_pool.tile([128, H], F32R, tag=tag)
            nc.tensor.transpose(vp, vT_sb[:, ts(t, 128)], ident_sb[0:64, 0:64])
            nc.vector.tensor_copy(v_sb[:, t, 0:H], vp)

    if probe == "A":
        # timing probe: stop after phase A; dump q_sb bytes as the output
        with tc.tile_pool(name="vtr" + rep, bufs=2, space="PSUM") as vtr:
            emit_vtrans(vtr, "vtr" + rep)
        nc.sync.dma_start(
            out=out.rearrange("(a p) h -> p a h", p=128),
            in_=v_sb[:, :, 0:H].bitcast(F32),
        )
        return

    # ---------------- Phase B: attention ----------------
    sc_pool = pool("scps", bufs=2, space="PSUM")
    ot_pool = pool("otps", bufs=2, space="PSUM")
    es_pool = pool("es", bufs=4)
    osb_pool = pool("osb", bufs=2)
    rec_pool = pool("rec", bufs=4)
    out_pool = pool("outsb", bufs=2)

    out_v = out.rearrange("(qb t p) h -> qb p t h", p=128, t=QT)

    acc = None
    if probe in ("AP", "ABX"):
        acc = singles.tile([128, 2], F32)
        nc.vector.memset(acc, 0.0)

    for qb in range(NQB):
        q0 = qb * QB
        oT_ps = None

        def emit_pv(k, es):
            for h in range(QB // 512):
                nc.tensor.matmul(
                    oT_ps[:, ts(h, 512)],
                    v_sb[:, k, :],
                    es[:, ts(h, 512)],
                    start=(k == 0),
                    stop=(k == KC - 1),
                )

        # software-pipelined: PE runs scores(k) while ACT exps k-1, then
        # PV(k-1).  The V^T->V transposes are slotted behind the first
        # scores/exp rounds (first q-block only) so they fill PE's ACT-lag
        # window instead of delaying the first scores matmul.
        prev = None
        for k in range(KC):
            sc = sc_pool.tile([128, QB], F32, tag="scps" + rep)  # 2 banks
            for h in range(QB // 512):
                qs = slice(q0 + h * 512, q0 + (h + 1) * 512)
                if pack_scores and k % 2 == 1:
                    # odd k runs in PE row-half B, concurrent with even k
                    nc.tensor.matmul(
                        sc[:, ts(h, 512)],
                        k2_sb[64:128, ts(k, 128)],
                        kq_sb[64:128, qs],
                        tile_position=(64, 0),
                    )
                else:
                    nc.tensor.matmul(
                        sc[:, ts(h, 512)],
                        kq_sb[0:64, ts(k, 128)],
                        q_sb[:, qs],
                    )
            if probe == "AP":
                nc.vector.tensor_add(acc[:, 0:1], acc[:, 1:2], sc[:, 0:1])
                continue
            es = es_pool.tile([128, QB], F32R)
            if probe == "ABX":
                nc.scalar.activation(
                    es, sc, AF.Exp, bias=mbias_sb[:, k : k + 1], scale=1.0,
                    accum_out=acc[:, 1:2],
                )
                nc.vector.tensor_add(acc[:, 0:1], acc[:, 0:1], acc[:, 1:2])
                continue
            nc.scalar.activation(
                es, sc, AF.Exp, bias=mbias_sb[:, k : k + 1], scale=1.0
            )
            if qb == 0 and k == 1:
                emit_vtrans(ot_pool, "otps" + rep)
            if probe == "AB":
                continue
            if prev is not None:
                if oT_ps is None:
                    oT_ps = ot_pool.tile([H + 1, QB], F32, tag="otps" + rep)
                emit_pv(*prev)
            prev = (k, es)
        if probe in ("AP", "ABX"):
            if qb == NQB - 1:
                nc.sync.dma_start(out=out[0:128, 0:2], in_=acc)
            continue
        if probe == "AB":
            if qb == NQB - 1:
                nc.sync.dma_start(
                    out=out.rearrange("(a p) h -> p a h", p=128),
                    in_=es.rearrange("p (a h) -> p a h", h=64)[:, 0:KC, :].bitcast(F32),
                )
            continue
        emit_pv(*prev)
        # transpose-mode matmul needs K in {32,64,128}: pad O'^T to 128
        # partitions (rows 65:127 zeroed; they land in unread out columns).
        # O-tail pipelined per 128-col tile: copy slice -> transpose ->
        # reciprocal -> scale -> (half-block) store.
        oT_s = osb_pool.tile([128, QB], F32R)
        nc.scalar.dma_start(out=oT_s[H : 128, :], in_=zeros64)
        ob = out_pool.tile([128, QT, H], F32)
        for t in range(QT):
            nc.vector.tensor_copy(
                oT_s[0 : H + 1, ts(t, 128)], oT_ps[:, ts(t, 128)]
            )
            op = ot_pool.tile([128, 128], F32R, tag="otps" + rep)
            nc.tensor.transpose(op, oT_s[:, ts(t, 128)], ident_sb)
            rec = rec_pool.tile([128, 1], F32)
            nc.vector.reciprocal(rec, op[:, H : H + 1])
            nc.vector.tensor_scalar_mul(ob[:, t, :], op[:, 0:H].bitcast(F32), rec)
            if t == QT // 2 - 1:
                nc.sync.dma_start(
                    out=out_v[qb][:, 0 : QT // 2, :], in_=ob[:, 0 : QT // 2, :]
                )
        nc.sync.dma_start(
            out=out_v[qb][:, QT // 2 : QT, :], in_=ob[:, QT // 2 : QT, :]
        )


def split_multi_waits(nc):
    """This walrus build encodes at most ONE sync-wait per hw instruction.
    Hoist all but the last wait of any multi-wait instruction into standalone
    single-wait NoOps on the same engine queue (semantically identical:
    engine-queue execution is in-order)."""
    import bass_rust

    ctr = 0
    for blk in nc.m.functions[0].blocks:
        insts = blk.instructions
        out = []
        changed = False
        for inst in insts:
            si = inst.sync_info
            if si is not None and si.on_wait and len(si.on_wait) > 1:
                waits = list(si.on_wait)
                for w in waits[:-1]:
                    ctr += 1
                    nop = mybir.InstNoOp(name=f"WSPLIT-{ctr}", ins=[], outs=[])
                    nop.engine = inst.engine
                    nop.sync_info = bass_rust.SyncInfo(on_wait=[w], on_update=[])
                    out.append(nop)
                inst.sync_info = bass_rust.SyncInfo(
                    on_wait=[waits[-1]], on_update=list(si.on_update or [])
                )
                out.append(inst)
                changed = True
            else:
                out.append(inst)
        if changed:
            insts[:] = out
    return nc


def build_bass(split=True, repeat=1, probe=None, pack_scores=False):
    nc = bass.Bass("TRN2", target_bir_lowering=False, debug=False)
    xT = nc.dram_tensor("xT", [W, S], F32R, kind="ExternalInput").ap()
    wkq = nc.dram_tensor("wkq", [128, WC * 128], F32R, kind="ExternalInput").ap()
    wv = nc.dram_tensor("wv", [128, WC * H], F32R, kind="ExternalInput").ap()
    bkq = nc.dram_tensor("bkq", [128, 1], F32, kind="ExternalInput").ap()
    bv = nc.dram_tensor("bv", [64, 1], F32, kind="ExternalInput").ap()
    ident = nc.dram_tensor("ident", [128, 128], F32R, kind="ExternalInput").ap()
    mbias = nc.dram_tensor("mbias", [128, KC], F32, kind="ExternalInput").ap()
    vones = nc.dram_tensor("vones", [128, KC], F32R, kind="ExternalInput").ap()
    zeros64 = nc.dram_tensor("zeros64", [H, QB], F32R, kind="ExternalInput").ap()
    out = nc.dram_tensor("out", [S, H], F32, kind="ExternalOutput").ap()
    with tile.TileContext(nc) as tc:
        for r in range(repeat):
            with ExitStack() as ctx:
                _emit(
                    ctx, tc, xT, wkq, wv, bkq, bv, ident, mbias, vones,
                    zeros64, out, rep=(f"_r{r}" if r else ""), probe=probe,
                    pack_scores=pack_scores,
                )
    if split:
        split_multi_waits(nc)
    return nc


def prep_in_maps(x, attn_mask, Wq, bq, Wk, bk, Wv, bv):
    x = np.asarray(x, dtype=np.float32)
    attn_mask = np.asarray(attn_mask)
    Wq = np.asarray(Wq, dtype=np.float32)
    Wk = np.asarray(Wk, dtype=np.float32)
    Wv = np.asarray(Wv, dtype=np.float32)
    bq = np.asarray(bq, dtype=np.float32)
    bk = np.asarray(bk, dtype=np.float32)
    bv = np.asarray(bv, dtype=np.float32)

    scale = np.float32(H) ** np.float32(-0.5)
    # [Wk | Wq*scale] -> per-w-chunk stationary layout [128, WC*128]
    wkq = np.concatenate([Wk, Wq * scale], axis=1)  # [W, 128]
    wkq = np.ascontiguousarray(
        wkq.reshape(WC, 128, 128).transpose(1, 0, 2).reshape(128, WC * 128)
    )
    wv_h = np.ascontiguousarray(
        Wv.reshape(WC, 128, H).transpose(1, 0, 2).reshape(128, WC * H)
    )
    bkq = np.concatenate([bk, bq * scale]).reshape(128, 1)
    bv_h = bv.reshape(H, 1)
    ident = np.eye(128, dtype=np.float32)

    in_maps = []
    for c in range(N_CORES):
        xT_c = np.ascontiguousarray(x[c].T)  # [W, S]
        m = attn_mask[c].astype(np.float32)  # [S]
        mb = np.where(m != 0, np.float32(0.0), np.float32(-1e30))
        mbias = np.ascontiguousarray(mb.reshape(KC, 128).T)  # [128, KC]
        in_maps.append(
            {
                "xT": xT_c,
                "wkq": wkq,
                "wv": wv_h,
                "bkq": np.ascontiguousarray(bkq),
                "bv": np.ascontiguousarray(bv_h),
                "ident": ident,
                "mbias": mbias,
                "vones": np.ones((128, KC), dtype=np.float32),
                "zeros64": np.zeros((H, QB), dtype=np.float32),
            }
        )
    return in_maps


def run(x, attn_mask, Wq, bq, Wk, bk, Wv, bv, trace=False, **rb_kwargs):
    from concourse.bass_utils import run_bass_kernel_spmd

    nc = build_bass(pack_scores=True)
    in_maps = prep_in_maps(x, attn_mask, Wq, bq, Wk, bk, Wv, bv)
    res = run_bass_kernel_spmd(
        nc, in_maps, core_ids=list(range(N_CORES)), trace=trace, **rb_kwargs
    )
    out = np.stack([r["out"] for r in res.results]).astype(np.float32)
    return out, res


def kernel(x, attn_mask, Wq, bq, Wk, bk, Wv, bv):
    out, _ = run(x, attn_mask, Wq, bq, Wk, bk, Wv, bv, trace=False)
    return out



# revision 21
# speedup vs baseline: 2.0581x; 2.0581x over previous
"""Single-head attention on 8 Trainium2 NeuronCores.

Problem: B=8, S=2048, WIDTH=1024, HEAD=64 single attention head.
Sharding: data-parallel over batch -- batch b runs on core b. No collectives.

v2: position-block pipelined, bf16 datapath.

x^T is host-prepped to bf16 and loaded in 8 column blocks of 256
positions (0.5 MB each).  As each block lands, K^T/Q^T (stacked, with
Wq pre-scaled) and V^T are projected for those positions, copied out
with biases on DVE, partition-remapped (Q^T down to 0:64, K^T up to
64:128) via DMA, and V^T chunks PE-transposed into V' (+ones column).
Attention units (k-chunk x query-half) are emitted interleaved with the
block loop as soon as their inputs exist, so the ACT exp chain (the
critical 33us of work) starts ~4us in, overlapping the remaining x DMA
and projections instead of following them.

Per unit (qh, k): scores^T = K^T_k.T @ Q^T[qh] ([128,1024] PSUM, even k
on PE row-half A, odd k row-half B via tile_position -- concurrent);
es = exp(scores^T + mask_bias) on ACT (bf16 out); O'^T[qh] += V'_k.T @ es
(PE, accumulating [65, 1024] PSUM; row 64 = softmax denominators from
the ones column).  O-tail per 128-query tile: bf16 staging copy,
PE-transpose, DVE reciprocal + scale, DMA out.
"""

import os
from contextlib import ExitStack

import numpy as np

import concourse.bass as bass
import concourse.tile as tile
from concourse import mybir
from concourse.bass import ts

S = 2048
W = 1024
H = 64
N_CORES = 8
WC = W // 128   # 8 w-chunks
KC = S // 128   # 16 k-chunks
NB = 8          # x position blocks
BP = S // NB    # 256 positions per block
QB = 1024       # query-half size
QT = QB // 128  # 8 q-tiles per half

F32 = mybir.dt.float32
BF16 = mybir.dt.bfloat16
AF = mybir.ActivationFunctionType

# score/exp/PV work is emitted as pairs (k-chunks kk, kk+1 on PE row
# halves A/B, back-to-back so they overlap) over a query range.  Early
# pairs use 512-wide query granules (qg = index in units of 512) so the
# ACT exp chain starts ~4.5us in, while x blocks 2..7 are still loading;
# once queries 0:1024 exist, pairs go 1024-wide (qh units).
# entries: (j_block -> list of (kind, qidx, kk)) kind 'g'=512-wide, 'p'=1024
PAIR_SCHED = {
    1: [("g", 0, 0)],
    2: [("g", 0, 2), ("g", 0, 4)],
    3: [("g", 1, 0), ("g", 1, 2), ("g", 1, 4)],
    4: [("p", 0, 6), ("p", 0, 8)],
    5: [("p", 0, 10)],
    6: [("p", 0, 12)],
    7: [("p", 0, 14)],
}


def _emit(ctx, tc, xb, wkq, wv, bkq, bv, identb, ident, mbias, vones, zpad,
          out, rep="", probe=None):
    nc = tc.nc

    def pool(name, **kw):
        return ctx.enter_context(tc.tile_pool(name=name + rep, **kw))

    singles = pool("singles", bufs=1)
    wkq_sb = singles.tile([128, WC * 128], BF16)
    nc.scalar.dma_start(out=wkq_sb, in_=wkq)
    wv_sb = singles.tile([128, WC * H], BF16)
    nc.scalar.dma_start(out=wv_sb, in_=wv)
    bkq_sb = singles.tile([128, 1], F32)
    nc.scalar.dma_start(out=bkq_sb, in_=bkq)
    bvb_sb = singles.tile([128, H], BF16)
    nc.scalar.dma_start(out=bvb_sb, in_=bv)
    identb_sb = singles.tile([128, H], BF16)
    nc.scalar.dma_start(out=identb_sb, in_=identb)
    ident_sb = singles.tile([128, 128], BF16)
    nc.scalar.dma_start(out=ident_sb, in_=ident)
    mbias_sb = singles.tile([128, KC], F32)
    nc.scalar.dma_start(out=mbias_sb, in_=mbias)
    warm_sb = singles.tile([1, 1], F32)
    nc.scalar.activation(warm_sb, mbias_sb[0:1, 0:1], AF.Exp,
                         bias=mbias_sb[0:1, 0:1], scale=1.0)

    kq_sb = singles.tile([128, S], BF16)   # rows 0:64 K^T, rows 64:128 Q^T
    q_sb = singles.tile([64, S], BF16)     # Q^T at partitions 0:64
    k2_sb = singles.tile([128, S], BF16)   # K^T replicated at partitions 64:128
    vT_sb = singles.tile([64, S], BF16)
    v_sb = singles.tile([128, KC, H + 1], BF16)  # V' chunks (+ones col)
    oTs_sb = singles.tile([128, 2, QB], BF16)    # O^T staging, rows 65:128 zero
    nc.gpsimd.dma_start(
        out=v_sb[:, :, H : H + 1],
        in_=vones.rearrange("p (k one) -> p k one", one=1),
    )
    nc.gpsimd.dma_start(
        out=oTs_sb[H + 1 : 128, :, :],
        in_=zpad.rearrange("p (a b) -> p a b", a=2),
    )

    # pools
    xp = pool("xp", bufs=3)
    kqvps = pool("kqvps", bufs=1, space="PSUM")
    scps = pool("scps", bufs=2, space="PSUM")
    otps = pool("otps", bufs=1, space="PSUM")
    trps = pool("trps", bufs=1, space="PSUM")
    es_pool = pool("es", bufs=6)
    rec_pool = pool("rec", bufs=4)
    ob_pool = pool("ob", bufs=2)

    out_v = out.rearrange("(qh t p) h -> qh p t h", p=128, t=QT)

    oT_ps = [None, None]
    ob_sb = [None, None]
    pending = []   # (q0, qw, k, es) awaiting their PV matmul
    started = set()  # (qh, h) oT 512-regions already start=True'd

    def flush_pv():
        for q0, qw, k, es in pending:
            qh = q0 // QB
            if oT_ps[qh] is None:
                ot_tile = otps.tile([H + 1, QB], F32, tag="otps" + rep)
                oT_ps[qh] = ot_tile
            for h in range(qw // 512):
                hreg = (q0 % QB) // 512 + h
                st = (qh, hreg) not in started
                started.add((qh, hreg))
                nc.tensor.matmul(
                    oT_ps[qh][:, ts(hreg, 512)], v_sb[:, k, :],
                    es[:, ts(h, 512)], start=st, stop=(k == KC - 1),
                )
        pending.clear()

    def emit_pair(q0, qw, kk, flush=True):
        sc0 = scps.tile([128, qw], F32, tag="scps" + rep)
        sc1 = scps.tile([128, qw], F32, tag="scps" + rep)
        for h in range(qw // 512):
            hs = slice(q0 + h * 512, q0 + (h + 1) * 512)
            nc.tensor.matmul(sc0[:, ts(h, 512)], kq_sb[0:64, ts(kk, 128)],
                             q_sb[:, hs])
            nc.tensor.matmul(
                sc1[:, ts(h, 512)], k2_sb[64:128, ts(kk + 1, 128)],
                kq_sb[64:128, hs], tile_position=(64, 0),
            )
        if flush:
            flush_pv()
        for k, sc in ((kk, sc0), (kk + 1, sc1)):
            es = es_pool.tile([128, qw], BF16)
            nc.scalar.activation(
                es, sc, AF.Exp, bias=mbias_sb[:, k : k + 1], scale=1.0
            )
            pending.append((q0, qw, k, es))

    def emit_sched(kind, qidx, kk):
        if kind == "g":
            emit_pair(qidx * 512, 512, kk)
        else:
            emit_pair(qidx * QB, QB, kk)

    def emit_otail_copies(qh):
        for h in range(2):
            nc.vector.tensor_copy(
                oTs_sb[0 : H + 1, qh, ts(h, 512)], oT_ps[qh][:, ts(h, 512)]
            )

    def emit_otail_rest(qh, t, ps_pool):
        if ob_sb[qh] is None:
            ob_tile = ob_pool.tile([128, QT, H], F32, tag="ob" + rep)
            ob_sb[qh] = ob_tile
        op = ps_pool.tile([128, 128], BF16, tag=ps_pool.name)
        nc.tensor.transpose(op, oTs_sb[:, qh, ts(t, 128)], ident_sb)
        rec = rec_pool.tile([128, 1], F32)
        nc.vector.reciprocal(rec, op[:, H : H + 1])
        nc.vector.tensor_scalar_mul(ob_sb[qh][:, t, :], op[:, 0:H], rec)
        if t % 4 == 3:
            nc.sync.dma_start(
                out=out_v[qh][:, t - 3 : t + 1, :],
                in_=ob_sb[qh][:, t - 3 : t + 1, :],
            )

    # ---------------- pipelined block loop ----------------
    for j in range(NB):
        xt = xp.tile([128, WC, BP], BF16)
        nc.sync.dma_start(out=xt, in_=xb[j].rearrange("p (g t) -> p g t", g=WC))
        pj_ps = kqvps.tile([128, 2 * BP], F32, tag="kqvps" + rep)
        kq_ps = pj_ps[:, 0:BP]
        vT_ps = pj_ps[0:64, BP : BP + BP]
        for g in range(WC):
            nc.tensor.matmul(
                kq_ps, wkq_sb[:, ts(g, 128)], xt[:, g, :],
                start=(g == 0), stop=(g == WC - 1),
            )
        for g in range(WC):
            nc.tensor.matmul(
                vT_ps, wv_sb[:, ts(g, H)], xt[:, g, :],
                start=(g == 0), stop=(g == WC - 1),
            )
        blk = slice(j * BP, (j + 1) * BP)
        nc.vector.tensor_scalar_add(kq_sb[:, blk], kq_ps, bkq_sb)
        nc.vector.tensor_copy(vT_sb[0:64, blk], vT_ps)
        nc.scalar.dma_start(out=q_sb[:, blk], in_=kq_sb[64:128, blk])
        nc.scalar.dma_start(out=k2_sb[64:128, blk], in_=kq_sb[0:64, blk])
        for kk in range(2 * j, 2 * j + 2):
            vp = trps.tile([128, H], BF16, tag="trps" + rep)
            nc.tensor.transpose(
                vp, vT_sb[:, ts(kk, 128)], ident_sb[0:64, 0:64]
            )
            nc.vector.scalar_tensor_tensor(
                v_sb[:, kk, 0:H], vp, 1.0, bvb_sb,
                mybir.AluOpType.mult, mybir.AluOpType.add,
            )
        if probe == "proj":
            continue
        for kind, qidx, kk in PAIR_SCHED.get(j, ()):
            emit_sched(kind, qidx, kk)

    if probe == "proj":
        nc.sync.dma_start(
            out=out.rearrange("(a p) h -> p a h", p=128),
            in_=kq_sb.bitcast(F32).rearrange("p (a h) -> p a h", h=H),
        )
        return

    # qh1 pairs; qh0's O^T must vacate the single otps buffer before
    # PV(1,0), so its staging copies are emitted (DVE) before that flush
    emit_pair(QB, QB, 0)       # flushes PV(0,14), PV(0,15)
    emit_pair(QB, QB, 2, flush=False)
    emit_otail_copies(0)       # frees oT_ps[0]
    for kk in range(4, KC, 2):
        emit_pair(QB, QB, kk)
        if kk <= 10:
            i = (kk - 4) // 2
            emit_otail_rest(0, 2 * i, trps)
            emit_otail_rest(0, 2 * i + 1, trps)
    # final pair region-major: finish oT region h fully, tail it, then h+1
    for h in range(2):
        for q0, qw, k, es in pending:
            nc.tensor.matmul(
                oT_ps[1][:, ts(h, 512)], v_sb[:, k, :], es[:, ts(h, 512)],
                start=False, stop=(k == KC - 1),
            )
        nc.vector.tensor_copy(
            oTs_sb[0 : H + 1, 1, ts(h, 512)], oT_ps[1][:, ts(h, 512)]
        )
        for t in range(4 * h, 4 * h + 4):
            emit_otail_rest(1, t, scps)
    pending.clear()


def split_multi_waits(nc):
    """This walrus build encodes at most ONE sync-wait per hw instruction.
    Hoist all but the last wait of any multi-wait instruction into standalone
    single-wait NoOps on the same engine queue (semantically identical:
    engine-queue execution is in-order)."""
    import bass_rust

    ctr = 0
    for blk in nc.m.functions[0].blocks:
        insts = blk.instructions
        out = []
        changed = False
        for inst in insts:
            si = inst.sync_info
            if si is not None and si.on_wait and len(si.on_wait) > 1:
                waits = list(si.on_wait)
                for w in waits[:-1]:
                    ctr += 1
                    nop = mybir.InstNoOp(name=f"WSPLIT-{ctr}", ins=[], outs=[])
                    nop.engine = inst.engine
                    nop.sync_info = bass_rust.SyncInfo(on_wait=[w], on_update=[])
                    out.append(nop)
                inst.sync_info = bass_rust.SyncInfo(
                    on_wait=[waits[-1]], on_update=list(si.on_update or [])
                )
                out.append(inst)
                changed = True
            else:
                out.append(inst)
        if changed:
            insts[:] = out
    return nc


def build_bass(split=True, repeat=1, probe=None, **_):
    nc = bass.Bass("TRN2", target_bir_lowering=False, debug=False)
    xbt = nc.dram_tensor("xb", [NB, 128, WC * BP], BF16, kind="ExternalInput").ap()
    wkq = nc.dram_tensor("wkq", [128, WC * 128], BF16, kind="ExternalInput").ap()
    wv = nc.dram_tensor("wv", [128, WC * H], BF16, kind="ExternalInput").ap()
    bkq = nc.dram_tensor("bkq", [128, 1], F32, kind="ExternalInput").ap()
    bv = nc.dram_tensor("bv", [128, H], BF16, kind="ExternalInput").ap()
    identb = nc.dram_tensor("identb", [128, H], BF16, kind="ExternalInput").ap()
    ident = nc.dram_tensor("ident", [128, 128], BF16, kind="ExternalInput").ap()
    mbias = nc.dram_tensor("mbias", [128, KC], F32, kind="ExternalInput").ap()
    vones = nc.dram_tensor("vones", [128, KC], BF16, kind="ExternalInput").ap()
    zpad = nc.dram_tensor("zpad", [128 - H - 1, 2 * QB], BF16,
                          kind="ExternalInput").ap()
    out = nc.dram_tensor("out", [S, H], F32, kind="ExternalOutput").ap()
    with tile.TileContext(nc) as tc:
        for r in range(repeat):
            with ExitStack() as ctx:
                _emit(
                    ctx, tc, xbt, wkq, wv, bkq, bv, identb, ident, mbias,
                    vones, zpad, out, rep=(f"_r{r}" if r else ""), probe=probe,
                )
    if split:
        split_multi_waits(nc)
    return nc


def prep_in_maps(x, attn_mask, Wq, bq, Wk, bk, Wv, bv):
    import ml_dtypes

    bf = ml_dtypes.bfloat16
    x = np.asarray(x, dtype=np.float32)
    attn_mask = np.asarray(attn_mask)
    Wq = np.asarray(Wq, dtype=np.float32)
    Wk = np.asarray(Wk, dtype=np.float32)
    Wv = np.asarray(Wv, dtype=np.float32)
    bq = np.asarray(bq, dtype=np.float32)
    bk = np.asarray(bk, dtype=np.float32)
    bv = np.asarray(bv, dtype=np.float32)

    scale = np.float32(H) ** np.float32(-0.5)
    # [Wk | Wq*scale] -> per-w-chunk stationary layout [128, WC*128]
    wkq = np.concatenate([Wk, Wq * scale], axis=1)  # [W, 128]
    wkq = np.ascontiguousarray(
        wkq.reshape(WC, 128, 128).transpose(1, 0, 2).reshape(128, WC * 128)
    ).astype(bf)
    wv_h = np.ascontiguousarray(
        Wv.reshape(WC, 128, H).transpose(1, 0, 2).reshape(128, WC * H)
    ).astype(bf)
    bkq = np.concatenate([bk, bq * scale]).reshape(128, 1)
    bv_h = np.broadcast_to(bv.reshape(1, H), (128, H)).astype(bf)
    ident = np.eye(128, dtype=np.float32).astype(bf)
    identb = np.ascontiguousarray(
        np.concatenate([np.eye(H), np.eye(H)], axis=0).astype(bf)
    )

    in_maps = []
    for c in range(N_CORES):
        # xb[j, p, g*BP+t] = x[c]^T[g*128+p, j*BP+t]
        xT_c = x[c].T.astype(bf)  # [W, S]
        xb = np.ascontiguousarray(
            xT_c.reshape(WC, 128, NB, BP).transpose(2, 1, 0, 3)
            .reshape(NB, 128, WC * BP)
        )
        m = attn_mask[c].astype(np.float32)  # [S]
        mb = np.where(m != 0, np.float32(0.0), np.float32(-1e30))
        mbias = np.ascontiguousarray(mb.reshape(KC, 128).T)  # [128, KC]
        in_maps.append(
            {
                "xb": xb,
                "wkq": wkq,
                "wv": wv_h,
                "bkq": np.ascontiguousarray(bkq),
                "bv": np.ascontiguousarray(bv_h),
                "identb": identb,
                "ident": ident,
                "mbias": mbias,
                "vones": np.ones((128, KC), dtype=np.float32).astype(bf),
                "zpad": np.zeros((128 - H - 1, 2 * QB), dtype=np.float32).astype(bf),
            }
        )
    return in_maps


def run(x, attn_mask, Wq, bq, Wk, bk, Wv, bv, trace=False, **rb_kwargs):
    from concourse.bass_utils import run_bass_kernel_spmd

    nc = build_bass()
    in_maps = prep_in_maps(x, attn_mask, Wq, bq, Wk, bk, Wv, bv)
    res = run_bass_kernel_spmd(
        nc, in_maps, core_ids=list(range(N_CORES)), trace=trace, **rb_kwargs
    )
    out = np.stack([r["out"] for r in res.results]).astype(np.float32)
    return out, res


def kernel(x, attn_mask, Wq, bq, Wk, bk, Wv, bv):
    out, _ = run(x, attn_mask, Wq, bq, Wk, bk, Wv, bv, trace=False)
    return out
